# revision 1
# baseline (speedup 1.0000x reference)
"""Trainium2 Bass kernel for nn_DeepCluster (vq_codebook).

Computation (per row x of shape [72]):
  7-layer MLP (Linear chain, ReLU after layers 2 and 4) -> f [200]
  sq[j]  = |f|^2 - 2*(f @ center)[j] + |center[:, j]|^2      (center: [200, 72])
  nom    = 1 / (1 + sq)                                       (alpha = 1)
  q      = nom / sum_j nom

Strategy: pure data parallel over 8 NeuronCores (batch split).  On each
core, activations flow as [features(partitions), batch(free)] bf16 tiles
of 512 rows; bf16 matmuls stream at 1 cycle/row on the PE with fast
weight load.  The distance computation runs transposed ([cluster,
batch]) so its matmuls also get a 512-wide moving operand; |c_j|^2 + 1
is added per-partition in f32 (it dominates sq, so it must not be
rounded to bf16), and the value path after the reciprocal stays f32.
PSUM->SBUF epilogues (bias add + optional ReLU) are split between the
Scalar and Vector engines.  The per-tile tail (reciprocal -> transpose
back -> row-normalize -> store) is software-pipelined one tile behind
the matmul stage so the PE never waits on the DVE round trip.
"""

import numpy as np

DIMS = [72, 128, 256, 256, 512, 512, 512, 200]
RELU_LAYERS = {2, 4}  # 1-indexed layers followed by ReLU
N_CORES = 8
N_FULL = 262144
B = 512  # rows per pipeline tile
P = 128

_CACHE = {}


def _build(n_rows):
    import concourse.bass as bass
    import concourse.mybir as mybir
    from concourse import bacc
    from concourse.tile import TileContext
    from concourse.masks import make_identity

    f32 = mybir.dt.float32
    bf16 = mybir.dt.bfloat16
    AF = mybir.ActivationFunctionType
    AX = mybir.AxisListType
    ALU = mybir.AluOpType

    kc_l = [(DIMS[i] + 127) // 128 for i in range(7)]
    mc_l = [(DIMS[i + 1] + 127) // 128 for i in range(7)]

    nc = bacc.Bacc(None, target_bir_lowering=False, debug=False)
    x_d = nc.dram_tensor("x", [n_rows, 72], bf16, kind="ExternalInput")
    q_d = nc.dram_tensor("q", [n_rows, 72], f32, kind="ExternalOutput")
    w_d, b_d = [], []
    for l in range(7):
        din, dout = DIMS[l], DIMS[l + 1]
        w_d.append(
            nc.dram_tensor(
                f"w{l + 1}", [min(din, 128), kc_l[l] * dout], bf16, kind="ExternalInput"
            )
        )
        b_d.append(nc.dram_tensor(f"b{l + 1}", [128, mc_l[l]], f32, kind="ExternalInput"))
    cm2A_d = nc.dram_tensor("cm2A", [128, 72], bf16, kind="ExternalInput")
    cm2B_d = nc.dram_tensor("cm2B", [72, 72], bf16, kind="ExternalInput")
    csq1_d = nc.dram_tensor("csq1", [72, 1], f32, kind="ExternalInput")

    n_tiles = n_rows // B
    assert n_rows % B == 0
    C = B // P  # 128-row chunks per tile

    with TileContext(nc) as tc:
        with (
            tc.tile_pool(name="consts", bufs=1) as consts,
            tc.tile_pool(name="acts", bufs=3) as acts,
            tc.tile_pool(name="pmm", bufs=4, space="PSUM") as pmm,
            tc.tile_pool(name="ptp", bufs=1, space="PSUM") as ptp,
            tc.tile_pool(name="psd", bufs=2, space="PSUM") as psd,
            tc.tile_pool(name="ppq", bufs=1, space="PSUM") as ppq,
        ):
            ones = consts.tile([128, 72], bf16, tag="ones")
            nc.vector.memset(ones, 1.0)
            ident = consts.tile([128, 128], bf16, tag="ident")
            make_identity(nc, ident)
            identf = consts.tile([128, 128], f32, tag="identf")
            make_identity(nc, identf)
            cm2A = consts.tile([128, 72], bf16, tag="cm2A")
            nc.sync.dma_start(out=cm2A, in_=cm2A_d[:])
            cm2B = consts.tile([72, 72], bf16, tag="cm2B")
            nc.sync.dma_start(out=cm2B, in_=cm2B_d[:])
            csq1 = consts.tile([72, 1], f32, tag="csq1")
            nc.sync.dma_start(out=csq1, in_=csq1_d[:])
            w_sb, b_sb = [], []
            for l in range(7):
                wt = consts.tile(list(w_d[l].shape), bf16, tag=f"w{l}")
                nc.sync.dma_start(out=wt, in_=w_d[l][:])
                w_sb.append(wt)
                bt = consts.tile([128, mc_l[l]], f32, tag=f"bias{l}")
                nc.sync.dma_start(out=bt, in_=b_d[l][:])
                b_sb.append(bt)

            x_r = x_d[:].rearrange("(t c p) j -> t p c j", p=P, c=C)
            q_r = q_d[:].rearrange("(t s p) j -> t p s j", p=P, s=C)

            def stageX(t):
                """x load + transpose -> xT [72, B] bf16 in SBUF."""
                x_sb = acts.tile([P, C, 72], bf16, tag="x")
                nc.sync.dma_start(out=x_sb, in_=x_r[t])
                ptx = ptp.tile([72, B], bf16, tag="xtp")
                for c in range(C):
                    nc.tensor.transpose(
                        ptx[:, P * c : P * (c + 1)], x_sb[:, c, :], ident
                    )
                xT = acts.tile([72, B], bf16, tag="xT")
                nc.vector.tensor_copy(xT, ptx)
                return xT

            def stageM(t, xT, next_xT_cb):
                """MLP + g + distance matmuls -> sdT PSUM.  Emits the next
                tile's input transposes mid-chain so the PE has filler work
                at layer-boundary epilogue stalls."""
                h = [xT]
                ep = 0
                for l in range(7):
                    dout = DIMS[l + 1]
                    kc, mc = kc_l[l], mc_l[l]
                    relu = (l + 1) in RELU_LAYERS
                    hn = []
                    for m in range(mc):
                        pw = min(128, dout - 128 * m)
                        ps = pmm.tile([pw, B], f32, tag="mm")
                        for k in range(kc):
                            lhsT = w_sb[l][:, k * dout + 128 * m : k * dout + 128 * m + pw]
                            nc.tensor.matmul(
                                ps, lhsT, h[k], start=(k == 0), stop=(k == kc - 1)
                            )
                        ht = acts.tile([pw, B], bf16, tag=f"h{l + 1}m{m}")
                        bias_col = b_sb[l][:pw, m : m + 1]
                        if ep % 2 == 0:  # scalar engine (ACT)
                            nc.scalar.activation(
                                out=ht,
                                in_=ps,
                                func=AF.Relu if relu else AF.Identity,
                                bias=bias_col,
                                scale=1.0,
                            )
                        else:  # vector engine (DVE)
                            if relu:
                                nc.vector.tensor_scalar(
                                    out=ht,
                                    in0=ps,
                                    scalar1=bias_col,
                                    scalar2=0.0,
                                    op0=ALU.add,
                                    op1=ALU.max,
                                )
                            else:
                                nc.vector.tensor_scalar_add(ht, ps, bias_col)
                        ep += 1
                        hn.append(ht)
                    h = hn
                    if l == 1 and next_xT_cb is not None:
                        next_xT_cb()

                f0, f1 = h  # [128, B], [72, B] bf16
                g0 = acts.tile([128, B], bf16, tag="g0")
                nc.vector.tensor_mul(g0, f0, f0)
                g1 = acts.tile([72, B], bf16, tag="g1")
                nc.vector.tensor_mul(g1, f1, f1)

                sdT = psd.tile([72, B], f32, tag="sd")
                nc.tensor.matmul(sdT, ones[:128, :72], g0, start=True, stop=False)
                nc.tensor.matmul(sdT, ones[:72, :72], g1, start=False, stop=False)
                nc.tensor.matmul(sdT, cm2A, f0, start=False, stop=False)
                nc.tensor.matmul(sdT, cm2B, f1, start=False, stop=True)
                return sdT

            def stageB(t, sdT):
                """csq add + reciprocal + transpose back + normalize + store."""
                sd1 = acts.tile([72, B], f32, tag="sd1")
                nc.scalar.activation(
                    out=sd1, in_=sdT, func=AF.Identity, bias=csq1[:, 0:1], scale=1.0
                )
                nomT = acts.tile([72, B], f32, tag="nomT")
                nc.vector.reciprocal_approx_fast(out=nomT, in_=sd1)

                pq = ppq.tile([P, C, 72], f32, tag="pq")
                for s in range(C):
                    nc.tensor.transpose(
                        pq[:, s, :], nomT[:, P * s : P * (s + 1)], identf[:72, :72]
                    )
                rs4 = acts.tile([P, C], f32, tag="rs4")
                nc.vector.reduce_sum(rs4, pq, axis=AX.X)
                rr4 = acts.tile([P, C], f32, tag="rr4")
                nc.vector.reciprocal(rr4, rs4)
                rr_b = bass.AP(
                    tensor=rr4.tensor,
                    offset=rr4.offset,
                    ap=[rr4.ap[0], rr4.ap[1], [0, 72]],
                )
                qt = acts.tile([P, C, 72], f32, tag="qt")
                nc.vector.tensor_tensor(out=qt, in0=pq, in1=rr_b, op=ALU.mult)
                nc.sync.dma_start(out=q_r[t], in_=qt)

            prev = None
            next_xT = [stageX(0)]

            for t in range(n_tiles):

                def make_cb(tn):
                    if tn >= n_tiles:
                        return None

                    def cb():
                        next_xT.append(stageX(tn))

                    return cb

                cur = (t, stageM(t, next_xT.pop(0), make_cb(t + 1)))
                if prev is not None:
                    stageB(*prev)
                prev = cur
            stageB(*prev)

    nc.compile()
    return nc


def _prep_consts(ws, bs, center):
    """Host-side marshalling of the small replicated weights."""
    import ml_dtypes

    bf = ml_dtypes.bfloat16
    kc_l = [(DIMS[i] + 127) // 128 for i in range(7)]
    mc_l = [(DIMS[i + 1] + 127) // 128 for i in range(7)]
    consts = {}
    for l in range(7):
        din, dout = DIMS[l], DIMS[l + 1]
        w = np.ascontiguousarray(ws[l], dtype=np.float32)
        if din > 128:
            kc = kc_l[l]
            w = np.ascontiguousarray(
                w.reshape(kc, 128, dout).transpose(1, 0, 2).reshape(128, kc * dout)
            )
        consts[f"w{l + 1}"] = w.astype(bf)
        bt = np.zeros((128, mc_l[l]), dtype=np.float32)
        for m in range(mc_l[l]):
            pw = min(128, dout - 128 * m)
            bt[:pw, m] = bs[l][128 * m : 128 * m + pw]
        consts[f"b{l + 1}"] = bt
    c = np.asarray(center, dtype=np.float32)
    consts["cm2A"] = np.ascontiguousarray(-2.0 * c[:128, :]).astype(bf)
    consts["cm2B"] = np.ascontiguousarray(-2.0 * c[128:, :]).astype(bf)
    consts["csq1"] = np.ascontiguousarray(
        (1.0 + (c.astype(np.float64) ** 2).sum(axis=0)).reshape(72, 1)
    ).astype(np.float32)
    return consts


def kernel(
    inputs, w1, b1, w2, b2, w3, b3, w4, b4, w5, b5, w6, b6, w7, b7, center
):
    import ml_dtypes
    from concourse.bass_utils import run_bass_kernel_spmd

    x = np.asarray(inputs).astype(ml_dtypes.bfloat16)
    n = x.shape[0]
    n_loc = n // N_CORES
    key = n_loc
    if key not in _CACHE:
        _CACHE[key] = _build(n_loc)
    nc = _CACHE[key]

    consts = _prep_consts(
        [w1, w2, w3, w4, w5, w6, w7], [b1, b2, b3, b4, b5, b6, b7], center
    )
    in_maps = []
    for c in range(N_CORES):
        m = {"x": np.ascontiguousarray(x[c * n_loc : (c + 1) * n_loc])}
        m.update(consts)
        in_maps.append(m)
    res = run_bass_kernel_spmd(nc, in_maps, core_ids=list(range(N_CORES)))
    return np.concatenate([res.results[c]["q"] for c in range(N_CORES)], axis=0)



# revision 17
# speedup vs baseline: 2.4878x; 2.4878x over previous
"""Trainium2 Bass kernel for nn_DeepCluster (vq_codebook).

Computation (per row x of shape [72]):
  7-layer MLP (Linear chain, ReLU after layers 2 and 4) -> f [200]
  sq[j]  = |f|^2 - 2*(f @ center)[j] + |center[:, j]|^2      (center: [200, 72])
  nom    = 1 / (1 + sq)                                       (alpha = 1)
  q      = nom / sum_j nom

Key algebraic optimization: ReLU only follows layers 2 and 4, so the 7
Linear layers collapse into 3 effective layers computed host-side in f64:
  E = W1@W2 [72, 256],  C = W3@W4 [256, 512],  D = W5@W6@W7 [512, 200]
(biases fold likewise).  E runs in bf16; C and D run as fp8 (e4m3)
DoubleRow matmuls (K=256 per pass, 2x PE throughput).  Power-of-2 scales
keep fp8 operands in range; the epilogues fold the descale, bias, and
ReLU into single ACT/DVE/GpSimd ops.

The distance runs with f as the *stationary* operand so the output lands
directly in [row, cluster] orientation (no fp32 transpose back):
  psd[n, j] = sum_k fT[k, n] * (-2c)[k, j]  (+ extra contraction rows:
  |f_n|^2 * ones_j and ones_n * (csq_hi + csq_lo))
csq = 1 + |c_j|^2 is split into a bf16 hi part and a bf16 residual so the
moving-operand rounding does not corrupt sq (csq ~ 200 dominates).
Row-normalization then reduces along the free axis.

Pure data parallel over 8 NeuronCores (batch split).
"""

import numpy as np

N_CORES = 8
N_FULL = 262144
B = 512  # rows per pipeline tile
P = 128

SA_C = 32.0     # scale of h2 (input to C) in fp8
SW_C = 512.0    # scale of C weights in fp8
SA_D = 128.0    # scale of h3 (input to D) in fp8
SW_D = 2048.0   # scale of D weights in fp8

_CACHE = {}


def _build(n_rows):
    import concourse.bass as bass
    import concourse.mybir as mybir
    from concourse import bacc
    from concourse.tile import TileContext
    from concourse.masks import make_identity

    f32 = mybir.dt.float32
    bf16 = mybir.dt.bfloat16
    fp8 = mybir.dt.float8e4
    AF = mybir.ActivationFunctionType
    AX = mybir.AxisListType
    ALU = mybir.AluOpType
    DR = mybir.MatmulPerfMode.DoubleRow

    scaleC = SA_D / (SA_C * SW_C)
    scaleD = 1.0 / (SA_D * SW_D)

    nc = bacc.Bacc(None, target_bir_lowering=False, debug=False)
    x_d = nc.dram_tensor("x", [n_rows, 72], bf16, kind="ExternalInput")
    q_d = nc.dram_tensor("q", [n_rows, 72], f32, kind="ExternalOutput")
    Eb_d = nc.dram_tensor("Eb", [72, 256], bf16, kind="ExternalInput")
    be_d = nc.dram_tensor("be", [128, 2], f32, kind="ExternalInput")
    C8_d = nc.dram_tensor("C8", [128, 2, 512], fp8, kind="ExternalInput")
    bc_d = nc.dram_tensor("bc", [128, 4], f32, kind="ExternalInput")
    D8_d = nc.dram_tensor("D8", [128, 4, 208], fp8, kind="ExternalInput")
    bd_d = nc.dram_tensor("bd", [128, 2], f32, kind="ExternalInput")
    cdA_d = nc.dram_tensor("cdA", [128, 72], bf16, kind="ExternalInput")
    cdB_d = nc.dram_tensor("cdB", [74, 72], bf16, kind="ExternalInput")

    n_tiles = n_rows // B
    assert n_rows % B == 0
    C = B // P  # 128-row chunks per tile

    with TileContext(nc) as tc:
        with (
            tc.tile_pool(name="consts", bufs=1) as consts,
            tc.tile_pool(name="acts", bufs=3) as acts,
            tc.tile_pool(name="ptp", bufs=1, space="PSUM") as ptp,
            tc.tile_pool(name="pmm", bufs=5, space="PSUM") as pmm,
            tc.tile_pool(name="psd", bufs=2, space="PSUM") as psd_p,
        ):
            ident = consts.tile([128, 128], bf16, tag="ident")
            make_identity(nc, ident)
            ones1 = consts.tile([128, 1], bf16, tag="ones1")
            nc.vector.memset(ones1, 1.0)
            Eb = consts.tile([72, 256], bf16, tag="Eb")
            nc.sync.dma_start(out=Eb, in_=Eb_d[:])
            be = consts.tile([128, 2], f32, tag="be")
            nc.sync.dma_start(out=be, in_=be_d[:])
            C8 = consts.tile([128, 2, 512], fp8, tag="C8")
            nc.sync.dma_start(out=C8, in_=C8_d[:])
            bc = consts.tile([128, 4], f32, tag="bc")
            nc.sync.dma_start(out=bc, in_=bc_d[:])
            D8 = consts.tile([128, 4, 208], fp8, tag="D8")
            nc.sync.dma_start(out=D8, in_=D8_d[:])
            bd = consts.tile([128, 2], f32, tag="bd")
            nc.sync.dma_start(out=bd, in_=bd_d[:])
            cdA = consts.tile([128, 72], bf16, tag="cdA")
            nc.sync.dma_start(out=cdA, in_=cdA_d[:])
            cdB = consts.tile([74, 72], bf16, tag="cdB")
            nc.sync.dma_start(out=cdB, in_=cdB_d[:])
            onesj = consts.tile([1, 72], bf16, tag="onesj")
            nc.vector.memset(onesj, 1.0)

            x_r = x_d[:].rearrange("(t c p) j -> t p c j", p=P, c=C)
            q_r = q_d[:].rearrange("(t s p) j -> t p s j", p=P, s=C)

            def head(t):
                """x load + transpose -> xT, E matmuls + epi -> h2 fp8."""
                x_sb = acts.tile([P, C, 72], bf16, tag="x")
                nc.sync.dma_start(out=x_sb, in_=x_r[t])
                ptx = ptp.tile([72, B], bf16, tag="xtp")
                for c in range(C):
                    nc.tensor.transpose(
                        ptx[:, P * c : P * (c + 1)], x_sb[:, c, :], ident
                    )
                xT = acts.tile([72, B], bf16, tag="xT")
                nc.vector.tensor_copy(xT, ptx)
                h2 = acts.tile([128, 2, B], fp8, tag="h2")
                for m in range(2):
                    ps = pmm.tile([128, B], f32, tag="mm")
                    nc.tensor.matmul(
                        ps, Eb[:, 128 * m : 128 * (m + 1)], xT, start=True, stop=True
                    )
                    # h2 = relu(psum + be) (E weights pre-scaled by SA_C)
                    nc.scalar.activation(
                        out=h2[:, m, :],
                        in_=ps,
                        func=AF.Relu,
                        bias=be[:, m : m + 1],
                        scale=1.0,
                    )
                return h2

            def midC(t, h2):
                """C DoubleRow matmuls + ACT epi -> h3 fp8, D matmuls -> psums."""
                h3 = acts.tile([128, 4, B], fp8, tag="h3")
                for m in range(4):
                    ps = pmm.tile([128, B], f32, tag="mm")
                    nc.tensor.matmul(
                        ps,
                        C8[:, :, 128 * m : 128 * (m + 1)],
                        h2,
                        start=True,
                        stop=True,
                        perf_mode=DR,
                    )
                    nc.scalar.activation(
                        out=h3[:, m, :],
                        in_=ps,
                        func=AF.Relu,
                        bias=bc[:, m : m + 1],
                        scale=scaleC,
                    )
                # m=1 runs 74 wide: weight cols 200-201 are zero-padded, and
                # their bias is 1.0, so the epilogue emits exact ones rows
                # that pair with the csq_hi/csq_lo rows of cdB.
                psD = []
                for m, pw in ((0, 128), (1, 74)):
                    ps = pmm.tile([pw, B], f32, tag="mm")
                    for k in range(2):
                        nc.tensor.matmul(
                            ps,
                            D8[:, 2 * k : 2 * k + 2, 128 * m : 128 * m + pw],
                            h3[:, 2 * k : 2 * k + 2, :],
                            start=(k == 0),
                            stop=(k == 1),
                            perf_mode=DR,
                        )
                    psD.append(ps)
                return psD

            def midD_pre(t, psD):
                """f epi (DVE) + squares (GpSimd, SBUF only)."""
                f0 = acts.tile([128, B], bf16, tag="f0")
                f1 = acts.tile([74, B], bf16, tag="f1")
                nc.vector.tensor_scalar(
                    out=f0, in0=psD[0], scalar1=scaleD, scalar2=bd[:, 0:1],
                    op0=ALU.mult, op1=ALU.add,
                )
                nc.vector.tensor_scalar(
                    out=f1, in0=psD[1], scalar1=scaleD,
                    scalar2=bd[:74, 1:2], op0=ALU.mult, op1=ALU.add,
                )
                g0 = acts.tile([128, B], bf16, tag="g0")
                nc.gpsimd.tensor_mul(g0, f0, f0)
                g1 = acts.tile([72, B], bf16, tag="g1")
                nc.gpsimd.tensor_mul(g1, f1[:72, :], f1[:72, :])
                return f0, f1, g0, g1

            def midD_mm(t, fg):
                """|f|^2 matmuls + distance matmuls -> psd (PE, one stage late)."""
                f0, f1, g0, g1 = fg
                psf = pmm.tile([1, B], f32, tag="mm")
                nc.tensor.matmul(psf, ones1[:128, :], g0, start=True, stop=False)
                nc.tensor.matmul(psf, ones1[:72, :], g1, start=False, stop=True)
                fsq = acts.tile([1, B], bf16, tag="fsq")
                nc.vector.tensor_copy(fsq, psf)
                sd = psd_p.tile([128, C, 72], f32, tag="sd")
                for s in range(C):
                    nc.tensor.matmul(
                        sd[:, s, :], f0[:, P * s : P * (s + 1)], cdA,
                        start=True, stop=False,
                    )
                    nc.tensor.matmul(
                        sd[:, s, :], f1[:, P * s : P * (s + 1)], cdB,
                        start=False, stop=False,
                    )
                    nc.tensor.matmul(
                        sd[:, s, :], fsq[:, P * s : P * (s + 1)], onesj,
                        start=False, stop=True,
                    )
                return sd

            def tail(t, sd):
                """reciprocal -> row-normalize -> store."""
                nom = acts.tile([128, C, 72], f32, tag="nom")
                nc.vector.reciprocal_approx_fast(out=nom, in_=sd)
                rs = acts.tile([128, C], f32, tag="rs")
                nc.vector.reduce_sum(rs, nom, axis=AX.X)
                rr = acts.tile([128, C], f32, tag="rr")
                nc.vector.reciprocal(rr, rs)
                rr_b = bass.AP(
                    tensor=rr.tensor,
                    offset=rr.offset,
                    ap=[rr.ap[0], rr.ap[1], [0, 72]],
                )
                qt = acts.tile([128, C, 72], f32, tag="qt")
                nc.vector.tensor_tensor(out=qt, in0=nom, in1=rr_b, op=ALU.mult)
                nc.sync.dma_start(out=q_r[t], in_=qt)

            # Software pipeline: the PE queue runs
            #   ... D(t) | xpose+E(t+1) | C(t+1)+D(t+1) | psf+dist(t) | ...
            # so the f->g->|f|^2 round trip of tile t (DVE+GpSimd) hides
            # behind tile t+1's matmuls instead of stalling the PE.
            h2_cur = head(0)
            psD_cur = midC(0, h2_cur)
            fg_cur = None
            prev_sd = None
            for t in range(n_tiles):
                h2_next = head(t + 1) if t + 1 < n_tiles else None
                fg_cur = midD_pre(t, psD_cur)
                if h2_next is not None:
                    psD_cur = midC(t + 1, h2_next)
                sd_t = midD_mm(t, fg_cur)
                if prev_sd is not None:
                    tail(t - 1, prev_sd)
                prev_sd = sd_t
            tail(n_tiles - 1, prev_sd)

    nc.compile()
    return nc


def _prep_consts(ws, bs, center):
    """Host-side f64 fusion of the 7 Linear layers into E, C, D + scaling."""
    import ml_dtypes

    bf = ml_dtypes.bfloat16
    f8 = ml_dtypes.float8_e4m3
    ws = [np.asarray(w, np.float64) for w in ws]
    bs = [np.asarray(b, np.float64) for b in bs]
    c = np.asarray(center, np.float64)

    E = ws[0] @ ws[1]
    be = bs[0] @ ws[1] + bs[1]
    Cm = ws[2] @ ws[3]
    bc = bs[2] @ ws[3] + bs[3]
    D = ws[4] @ ws[5] @ ws[6]
    bd = bs[4] @ ws[5] @ ws[6] + bs[5] @ ws[6] + bs[6]

    consts = {}
    consts["Eb"] = np.ascontiguousarray(E * SA_C).astype(bf)
    be2 = np.zeros((128, 2), np.float32)
    for m in range(2):
        be2[:, m] = (be[128 * m : 128 * (m + 1)] * SA_C).astype(np.float32)
    consts["be"] = be2
    consts["C8"] = np.ascontiguousarray(
        (Cm * SW_C).reshape(2, 128, 512).transpose(1, 0, 2)
    ).astype(f8)
    bc2 = np.zeros((128, 4), np.float32)
    for m in range(4):
        bc2[:, m] = (bc[128 * m : 128 * (m + 1)] * SA_D).astype(np.float32)
    consts["bc"] = bc2
    D8 = np.zeros((128, 4, 208), np.float64)
    D8[:, :, :200] = (D * SW_D).reshape(4, 128, 200).transpose(1, 0, 2)
    consts["D8"] = np.ascontiguousarray(D8).astype(f8)
    bd2 = np.zeros((128, 2), np.float32)
    bd2[:, 0] = bd[:128].astype(np.float32)
    bd2[:72, 1] = bd[128:].astype(np.float32)
    bd2[72:74, 1] = 1.0  # ones rows of f1 (psum is 0 there: zero-padded D8)
    consts["bd"] = bd2
    consts["cdA"] = np.ascontiguousarray(-2.0 * c[:128, :]).astype(bf)
    cdB = np.zeros((74, 72), np.float64)
    cdB[:72, :] = -2.0 * c[128:, :]
    csq1 = 1.0 + (c * c).sum(axis=0)
    csq_hi = csq1.astype(bf).astype(np.float64)
    cdB[72, :] = csq_hi
    cdB[73, :] = csq1 - csq_hi
    consts["cdB"] = cdB.astype(bf)
    return consts


def kernel(
    inputs, w1, b1, w2, b2, w3, b3, w4, b4, w5, b5, w6, b6, w7, b7, center
):
    import ml_dtypes
    from concourse.bass_utils import run_bass_kernel_spmd

    x = np.asarray(inputs).astype(ml_dtypes.bfloat16)
    n = x.shape[0]
    n_loc = n // N_CORES
    key = n_loc
    if key not in _CACHE:
        _CACHE[key] = _build(n_loc)
    nc = _CACHE[key]

    consts = _prep_consts(
        [w1, w2, w3, w4, w5, w6, w7], [b1, b2, b3, b4, b5, b6, b7], center
    )
    in_maps = []
    for c in range(N_CORES):
        m = {"x": np.ascontiguousarray(x[c * n_loc : (c + 1) * n_loc])}
        m.update(consts)
        in_maps.append(m)
    res = run_bass_kernel_spmd(nc, in_maps, core_ids=list(range(N_CORES)))
    return np.concatenate([res.results[c]["q"] for c in range(N_CORES)], axis=0)
